# revision 8
# baseline (speedup 1.0000x reference)
"""Chamfer distance kernel for Trainium2 (8 NeuronCores, SPMD).

Math: for point sets a[16384,3], b[16384,3],
  d2(i,j) = |a_i|^2 + |b_j|^2 - 2 a_i.b_j
encoded as a K=5 augmented inner product so the TensorEngine emits squared
distances directly:
  aug_a[:,i] = (-2ax, -2ay, -2az, 1, |a_i|^2)
  aug_b[:,j] = ( bx,   by,   bz, |b_j|^2, 1)
  dot(aug_a[:,i], aug_b[:,j]) = d2(i,j)

Sharding: a's rows split across 8 cores (2048 each); every core holds all
of b.  Each core computes, over its [2048, 16384] block:
  - row mins   (nearest b for each of its a points)       : a->b direction
  - col mins   (nearest of ITS a's for every b point)     : partial b->a
Both reductions are along the PSUM free axis: the a->b direction uses
matmuls with a as the stationary operand, the b->a direction re-computes
the block transposed (b stationary, a moving), which costs only cheap
TensorEngine time and avoids partition-axis reductions.

min/sqrt commute, so sqrt + the cross-core combine (elementwise min of the
8 partial b->a vectors + mean) happen on the host on 8*(2048+16384) floats.
"""

import numpy as np

N = 16384          # points in each set
D = 3
NCORES = 8
NS = N // NCORES   # a-rows per core = 2048
K = 5              # augmented contraction dim
P = 128            # partitions
MM_N = 512         # fp32 matmul max moving free dim
GRP = 2048         # psum group = 4 matmuls of 512 (4 banks)

_CACHE = {}


def _build_nc():
    from contextlib import ExitStack

    import concourse.bacc as bacc
    import concourse.mybir as mybir
    import concourse.tile as tile

    f32 = mybir.dt.float32
    AX = mybir.AxisListType.X
    MIN = mybir.AluOpType.min

    nc = bacc.Bacc()
    # Single fused input: cols [0,NS) = this core's aug-a shard, [NS,NS+N) = aug-b.
    # One tensor -> one DMA -> one wait semaphore on the first matmul (the
    # fp32 self-loading matmul's LDWEIGHTS struct only supports one sync wait).
    aug = nc.dram_tensor("aug", [K, NS + N], f32, kind="ExternalInput")
    # row_out[p, n] = min_j d2(a[n*128+p], b[j])     (this core's a-shard)
    # col_out[p, m] = min_{i in shard} d2(a_i, b[m*128+p])
    row_out = nc.dram_tensor("row_out", [P, NS // P], f32, kind="ExternalOutput")
    col_out = nc.dram_tensor("col_out", [P, N // P], f32, kind="ExternalOutput")

    with tile.TileContext(nc) as tc, ExitStack() as ctx:
        sb = ctx.enter_context(tc.tile_pool(name="sb", bufs=1))
        ps = ctx.enter_context(tc.tile_pool(name="ps", bufs=2, space="PSUM"))
        mn = ctx.enter_context(tc.tile_pool(name="mn", bufs=4))
        outp = ctx.enter_context(tc.tile_pool(name="outp", bufs=1))

        aug_sb = sb.tile([K, NS + N], f32)
        nc.sync.dma_start(out=aug_sb[:, :], in_=aug[:, :])
        a_sb = aug_sb[:, 0:NS]
        b_sb = aug_sb[:, NS:NS + N]

        row_acc = outp.tile([P, NS // P], f32)
        col_acc = outp.tile([P, N // P], f32)

        # ---- direction 1: a-shard rows on partitions, min over all b (free axis)
        n_chunks = NS // P              # 16
        m_groups = N // GRP             # 8
        for n in range(n_chunks):
            minlets = mn.tile([P, m_groups], f32, tag="minlets")
            for mg in range(m_groups):
                pt = ps.tile([P, GRP], f32, tag="pt")
                for k4 in range(GRP // MM_N):
                    nc.tensor.matmul(
                        pt[:, k4 * MM_N:(k4 + 1) * MM_N],
                        a_sb[:, n * P:(n + 1) * P],
                        b_sb[:, mg * GRP + k4 * MM_N: mg * GRP + (k4 + 1) * MM_N],
                        start=True,
                        stop=True,
                    )
                nc.vector.tensor_reduce(
                    minlets[:, mg:mg + 1], pt[:, :], axis=AX, op=MIN
                )
            nc.vector.tensor_reduce(
                row_acc[:, n:n + 1], minlets[:, :], axis=AX, op=MIN
            )
        nc.sync.dma_start(out=row_out[:, :], in_=row_acc[:, :])

        # ---- direction 2: b rows on partitions, min over this a-shard (free axis)
        mm_chunks = N // P              # 128
        for mm in range(mm_chunks):
            pt = ps.tile([P, GRP], f32, tag="pt")
            for k4 in range(NS // MM_N):
                nc.tensor.matmul(
                    pt[:, k4 * MM_N:(k4 + 1) * MM_N],
                    b_sb[:, mm * P:(mm + 1) * P],
                    a_sb[:, k4 * MM_N:(k4 + 1) * MM_N],
                    start=True,
                    stop=True,
                )
            nc.vector.tensor_reduce(
                col_acc[:, mm:mm + 1], pt[:, :], axis=AX, op=MIN
            )
        nc.sync.dma_start(out=col_out[:, :], in_=col_acc[:, :])

    nc.compile()
    return nc


def _get_nc():
    if "nc" not in _CACHE:
        _CACHE["nc"] = _build_nc()
    return _CACHE["nc"]


def _install_ntff_hook():
    """The agent image's `antenv` lacks `axon_hooks`; provide it so
    run_bass_kernel_spmd(trace=True) can profile via the axon PJRT .so."""
    import sys

    if "antenv.axon_hooks" in sys.modules:
        return
    try:
        import contextlib
        import ctypes
        import types

        so_path = "/opt/axon/libaxon_pjrt.so"
        lib = ctypes.CDLL(so_path)
        if not hasattr(lib, "axon_start_nrt_profile"):
            return
        lib.axon_start_nrt_profile.argtypes = [
            ctypes.POINTER(ctypes.c_int64),
            ctypes.c_size_t,
        ]
        lib.axon_start_nrt_profile.restype = ctypes.c_int64
        lib.axon_stop_nrt_profile.argtypes = [ctypes.c_char_p]
        lib.axon_stop_nrt_profile.restype = ctypes.c_int64

        @contextlib.contextmanager
        def _hook(output_dir, device_ids):
            import jax

            jax.devices()
            if device_ids:
                ids = (ctypes.c_int64 * len(device_ids))(*device_ids)
                rc = lib.axon_start_nrt_profile(ids, len(device_ids))
            else:
                rc = lib.axon_start_nrt_profile(None, 0)
            if rc != 0:
                raise RuntimeError(f"axon_start_nrt_profile rc={rc}")
            try:
                yield
            finally:
                n = lib.axon_stop_nrt_profile(str(output_dir).encode())
                if n < 0:
                    raise RuntimeError(f"axon_stop_nrt_profile rc={n}")

        mod = types.ModuleType("antenv.axon_hooks")
        mod.get_axon_ntff_profile_hook = lambda: _hook
        mod.set_axon_ntff_profile_hook = lambda h: None
        sys.modules["antenv.axon_hooks"] = mod
    except Exception:
        pass


def _run(in_maps, trace=False):
    from concourse.bass_utils import run_bass_kernel_spmd

    if trace:
        _install_ntff_hook()
    nc = _get_nc()
    res = run_bass_kernel_spmd(
        nc, in_maps, core_ids=list(range(NCORES)), trace=trace
    )
    _CACHE["last_exec_ns"] = res.exec_time_ns
    _CACHE["last_trace"] = res.instructions_and_trace
    return res.results


def kernel(a, b):
    a = np.ascontiguousarray(np.asarray(a, dtype=np.float32))
    b = np.ascontiguousarray(np.asarray(b, dtype=np.float32))
    assert a.shape == (N, D) and b.shape == (N, D), (a.shape, b.shape)

    a2 = np.sum(a.astype(np.float64) * a, axis=1).astype(np.float32)
    b2 = np.sum(b.astype(np.float64) * b, axis=1).astype(np.float32)

    aug_a = np.empty((K, N), np.float32)
    aug_a[0:3] = -2.0 * a.T
    aug_a[3] = 1.0
    aug_a[4] = a2
    aug_b = np.empty((K, N), np.float32)
    aug_b[0:3] = b.T
    aug_b[3] = b2
    aug_b[4] = 1.0

    import os
    trace = bool(int(os.environ.get("CHAMFER_TRACE", "0")))
    in_maps = [
        {
            "aug": np.ascontiguousarray(
                np.concatenate([aug_a[:, r * NS:(r + 1) * NS], aug_b], axis=1)
            ),
        }
        for r in range(NCORES)
    ]
    results = _run(in_maps, trace=trace)

    # row_out[p, n] -> row index i = n*128 + p ; shards in core order
    rows = np.concatenate(
        [results[r]["row_out"].T.reshape(-1) for r in range(NCORES)]
    )
    # col partials: min over cores
    cols = np.min(
        np.stack([results[r]["col_out"].T.reshape(-1) for r in range(NCORES)]),
        axis=0,
    )
    mins_sq = np.concatenate([rows, cols])
    dist = np.sqrt(np.maximum(mins_sq, 0.0))
    return np.asarray(np.mean(dist), dtype=np.float32)


# revision 9
# speedup vs baseline: 3.0008x; 3.0008x over previous
"""Chamfer distance kernel for Trainium2 (8 NeuronCores, SPMD).

Math: for point sets a[16384,3], b[16384,3],
  d2(i,j) = |a_i|^2 + |b_j|^2 - 2 a_i.b_j
encoded as an augmented inner product so the TensorEngine emits squared
distances directly.

fp32 matmuls on TRN2 are ~5x slower than bf16 (hi/lo double pass, ~5
cycles/column).  Instead each fp32 operand is split into three bf16 pieces
(value = h + m + l) and the products needed for ~fp32 accuracy are laid out
along the contraction axis: 8 piece-pairs per coordinate (hh, hm, mh, hl,
lh, mm, ml, lm; only l*l is dropped, ~2^-32 relative) = 24 rows, plus 3
rows for |b|^2 (paired with exact 1.0) and 3 for |a|^2.  K=30 <= 32, so a
single bf16 matmul per tile computes d2 at fp32-grade accuracy at bf16
speed (matmul cost scales with streamed columns, not K).

Sharding: a's rows split across 8 cores (2048 each); every core holds all
of b.  Each core computes over its [2048, 16384] block:
  - row mins (a->b direction): a stationary, b streaming
  - col mins (partial b->a): the block is re-computed transposed
    (b stationary, a streaming) so BOTH reductions are free-axis reduces.
min/sqrt commute; sqrt + cross-core combine (elementwise min of the 8
partial b->a vectors + mean) run on the host on 8*(2048+16384) floats.
"""

import numpy as np

N = 16384          # points in each set
D = 3
NCORES = 8
NS = N // NCORES   # a-rows per core = 2048
K = 30             # split-precision contraction rows
P = 128            # partitions
MM_N = 512         # matmul free dim per PSUM bank
GRP = 2048         # psum group = 4 matmuls of 512 (4 banks)

# column layout of the fused input tensor: [Wa shard | Rb | Wb | Ra shard]
OFF_WA = 0
OFF_RB = NS
OFF_WB = NS + N
OFF_RA = NS + N + N
TOT_COLS = 2 * (NS + N)

_CACHE = {}


def _build_nc():
    from contextlib import ExitStack

    import concourse.bacc as bacc
    import concourse.mybir as mybir
    import concourse.tile as tile

    bf16 = mybir.dt.bfloat16
    f32 = mybir.dt.float32
    AX = mybir.AxisListType.X
    MIN = mybir.AluOpType.min

    nc = bacc.Bacc()
    aug = nc.dram_tensor("aug", [K, TOT_COLS], bf16, kind="ExternalInput")
    # row_out[p, n] = min_j d2(a[n*128+p], b[j])     (this core's a-shard)
    # col_out[p, m] = min_{i in shard} d2(a_i, b[m*128+p])
    row_out = nc.dram_tensor("row_out", [P, NS // P], f32, kind="ExternalOutput")
    col_out = nc.dram_tensor("col_out", [P, N // P], f32, kind="ExternalOutput")

    with tile.TileContext(nc) as tc, ExitStack() as ctx:
        sb = ctx.enter_context(tc.tile_pool(name="sb", bufs=1))
        ps = ctx.enter_context(tc.tile_pool(name="ps", bufs=2, space="PSUM"))
        mn = ctx.enter_context(tc.tile_pool(name="mn", bufs=4))
        outp = ctx.enter_context(tc.tile_pool(name="outp", bufs=1))

        aug_sb = sb.tile([K, TOT_COLS], bf16)
        nc.sync.dma_start(out=aug_sb[:, :], in_=aug[:, :])

        row_acc = outp.tile([P, NS // P], f32)
        col_acc = outp.tile([P, N // P], f32)

        # ---- direction 1: a-shard rows on partitions, min over all b
        n_chunks = NS // P              # 16
        m_groups = N // GRP             # 8
        for n in range(n_chunks):
            minlets = mn.tile([P, m_groups], f32, tag="minlets")
            for mg in range(m_groups):
                pt = ps.tile([P, GRP], f32, tag="pt")
                for k4 in range(GRP // MM_N):
                    c0 = OFF_RB + mg * GRP + k4 * MM_N
                    nc.tensor.matmul(
                        pt[:, k4 * MM_N:(k4 + 1) * MM_N],
                        aug_sb[:, OFF_WA + n * P: OFF_WA + (n + 1) * P],
                        aug_sb[:, c0: c0 + MM_N],
                        start=True,
                        stop=True,
                    )
                nc.vector.tensor_reduce(
                    minlets[:, mg:mg + 1], pt[:, :], axis=AX, op=MIN
                )
            nc.vector.tensor_reduce(
                row_acc[:, n:n + 1], minlets[:, :], axis=AX, op=MIN
            )
        nc.sync.dma_start(out=row_out[:, :], in_=row_acc[:, :])

        # ---- direction 2: b rows on partitions, min over this a-shard
        mm_chunks = N // P              # 128
        for mm in range(mm_chunks):
            pt = ps.tile([P, GRP], f32, tag="pt")
            for k4 in range(NS // MM_N):
                c0 = OFF_RA + k4 * MM_N
                nc.tensor.matmul(
                    pt[:, k4 * MM_N:(k4 + 1) * MM_N],
                    aug_sb[:, OFF_WB + mm * P: OFF_WB + (mm + 1) * P],
                    aug_sb[:, c0: c0 + MM_N],
                    start=True,
                    stop=True,
                )
            nc.vector.tensor_reduce(
                col_acc[:, mm:mm + 1], pt[:, :], axis=AX, op=MIN
            )
        nc.sync.dma_start(out=col_out[:, :], in_=col_acc[:, :])

    nc.compile()
    return nc


def _get_nc():
    if "nc" not in _CACHE:
        _CACHE["nc"] = _build_nc()
    return _CACHE["nc"]


def _install_ntff_hook():
    """The agent image's `antenv` lacks `axon_hooks`; provide it so
    run_bass_kernel_spmd(trace=True) can profile via the axon PJRT .so."""
    import sys

    if "antenv.axon_hooks" in sys.modules:
        return
    try:
        import contextlib
        import ctypes
        import types

        so_path = "/opt/axon/libaxon_pjrt.so"
        lib = ctypes.CDLL(so_path)
        if not hasattr(lib, "axon_start_nrt_profile"):
            return
        lib.axon_start_nrt_profile.argtypes = [
            ctypes.POINTER(ctypes.c_int64),
            ctypes.c_size_t,
        ]
        lib.axon_start_nrt_profile.restype = ctypes.c_int64
        lib.axon_stop_nrt_profile.argtypes = [ctypes.c_char_p]
        lib.axon_stop_nrt_profile.restype = ctypes.c_int64

        @contextlib.contextmanager
        def _hook(output_dir, device_ids):
            import jax

            jax.devices()
            if device_ids:
                ids = (ctypes.c_int64 * len(device_ids))(*device_ids)
                rc = lib.axon_start_nrt_profile(ids, len(device_ids))
            else:
                rc = lib.axon_start_nrt_profile(None, 0)
            if rc != 0:
                raise RuntimeError(f"axon_start_nrt_profile rc={rc}")
            try:
                yield
            finally:
                n = lib.axon_stop_nrt_profile(str(output_dir).encode())
                if n < 0:
                    raise RuntimeError(f"axon_stop_nrt_profile rc={n}")

        mod = types.ModuleType("antenv.axon_hooks")
        mod.get_axon_ntff_profile_hook = lambda: _hook
        mod.set_axon_ntff_profile_hook = lambda h: None
        sys.modules["antenv.axon_hooks"] = mod
    except Exception:
        pass


def _run(in_maps, trace=False):
    from concourse.bass_utils import run_bass_kernel_spmd

    if trace:
        _install_ntff_hook()
    nc = _get_nc()
    res = run_bass_kernel_spmd(
        nc, in_maps, core_ids=list(range(NCORES)), trace=trace
    )
    _CACHE["last_exec_ns"] = res.exec_time_ns
    _CACHE["last_trace"] = res.instructions_and_trace
    return res.results


def _split3(x):
    """fp32 -> three stacked bf16 pieces (as fp32 for further math)."""
    import ml_dtypes

    h = x.astype(ml_dtypes.bfloat16).astype(np.float32)
    r = x - h
    m = r.astype(ml_dtypes.bfloat16).astype(np.float32)
    l = (r - m).astype(np.float32)
    return h, m, l


# piece-pair schedule per coordinate: indices into (h, m, l)
_PAIRS = [(0, 0), (0, 1), (1, 0), (0, 2), (2, 0), (1, 1), (1, 2), (2, 1)]


def _build_wr(Pts, Qts, P2, Q2):
    """W from the stationary set (with -2*coords and |P|^2), R from the
    streaming set (coords and |Q|^2), such that W[:, i] . R[:, j] = d2."""
    W = np.zeros((K, Pts.shape[0]), np.float32)
    R = np.zeros((K, Qts.shape[0]), np.float32)
    k = 0
    for d in range(D):
        u = _split3(-2.0 * Pts[:, d])
        v = _split3(Qts[:, d])
        for wp, rp in _PAIRS:
            W[k] = u[wp]
            R[k] = v[rp]
            k += 1
    q2p = _split3(Q2)
    for t in range(3):
        W[k] = 1.0
        R[k] = q2p[t]
        k += 1
    p2p = _split3(P2)
    for t in range(3):
        W[k] = p2p[t]
        R[k] = 1.0
        k += 1
    assert k == K
    return W, R


def kernel(a, b):
    import ml_dtypes

    a = np.ascontiguousarray(np.asarray(a, dtype=np.float32))
    b = np.ascontiguousarray(np.asarray(b, dtype=np.float32))
    assert a.shape == (N, D) and b.shape == (N, D), (a.shape, b.shape)

    a2 = np.sum(a.astype(np.float64) * a, axis=1).astype(np.float32)
    b2 = np.sum(b.astype(np.float64) * b, axis=1).astype(np.float32)

    Wa, Rb = _build_wr(a, b, a2, b2)   # direction 1: a stationary, b streaming
    Wb, Ra = _build_wr(b, a, b2, a2)   # direction 2: b stationary, a streaming

    import os
    trace = bool(int(os.environ.get("CHAMFER_TRACE", "0")))
    in_maps = []
    for r in range(NCORES):
        buf = np.empty((K, TOT_COLS), np.float32)
        buf[:, OFF_WA:OFF_WA + NS] = Wa[:, r * NS:(r + 1) * NS]
        buf[:, OFF_RB:OFF_RB + N] = Rb
        buf[:, OFF_WB:OFF_WB + N] = Wb
        buf[:, OFF_RA:OFF_RA + NS] = Ra[:, r * NS:(r + 1) * NS]
        in_maps.append({"aug": buf.astype(ml_dtypes.bfloat16)})
    results = _run(in_maps, trace=trace)

    # row_out[p, n] -> row index i = n*128 + p ; shards in core order
    rows = np.concatenate(
        [results[r]["row_out"].T.reshape(-1) for r in range(NCORES)]
    )
    # col partials: min over cores
    cols = np.min(
        np.stack([results[r]["col_out"].T.reshape(-1) for r in range(NCORES)]),
        axis=0,
    )
    mins_sq = np.concatenate([rows, cols])
    dist = np.sqrt(np.maximum(mins_sq, 0.0))
    return np.asarray(np.mean(dist), dtype=np.float32)


# revision 10
# speedup vs baseline: 3.7020x; 1.2337x over previous
"""Chamfer distance kernel for Trainium2 (8 NeuronCores, SPMD).

Math: for point sets a[16384,3], b[16384,3],
  d2(i,j) = |a_i|^2 + |b_j|^2 - 2 a_i.b_j
encoded as an augmented inner product so the TensorEngine emits squared
distances directly.

fp32 matmuls on TRN2 are ~5x slower than bf16 (hi/lo double pass).  Each
fp32 operand is instead split into three bf16 pieces (value = h + m + l)
and the piece-products needed for ~fp32 accuracy are laid out along the
contraction axis: 8 piece-pairs per coordinate (only l*l dropped) = 24
rows, plus 3 rows for |b|^2 and 3 for |a|^2.  K=30 <= 32, so one bf16
matmul per tile computes exact-enough d2 (matmul cost scales with streamed
columns, not K).

The K<=32 contraction also enables 4-way row-group packing: operands are
replicated at SBUF partition offsets 0/32/64/96 and 4 matmuls run
concurrently in disjoint 32-row groups of the PE array via tile_position,
quadrupling TensorEngine throughput.

Reductions: every [128,2048] fp32 PSUM group must be min-reduced along the
free axis.  The DVE reduces fp32 PSUM at 1 elem/lane/cycle only; to beat
that, a fraction of groups is "ACT-assisted": ScalarE copies PSUM ->
SBUF bf16 (1x on the otherwise idle Scalar engine) and the DVE min-folds
bf16 tiles pairwise at its 2x packed rate.  The assist fraction is chosen
so ScalarE and VectorE finish together.

Sharding: a's rows split across 8 cores (2048 each); every core holds all
of b.  Each core computes row mins of its [2048, 16384] block (a->b) and,
via re-computing the block transposed, col mins (partial b->a).  sqrt and
the cross-core combine (elementwise min of 8 partial vectors + mean) run
on the host on 8*(2048+16384) floats; min/sqrt commute.
"""

import numpy as np

N = 16384          # points in each set
D = 3
NCORES = 8
NS = N // NCORES   # a-rows per core = 2048
K = 30             # split-precision contraction rows
KPAD = 32          # row-group stride for replicas
P = 128            # partitions
MM_N = 512         # matmul free dim per PSUM bank
GRP = 2048         # psum group = 4 matmuls of 512 (4 banks)

# column layout of the fused input tensor: [Wa shard | Rb | Wb | Ra shard]
OFF_WA = 0
OFF_RB = NS
OFF_WB = NS + N
OFF_RA = NS + N + N
TOT_COLS = 2 * (NS + N)

# ACT-assist schedules (tuned so ScalarE busy ~= VectorE busy)
D1_DIRECT = {0, 4}          # m-groups per n-chunk reduced directly from PSUM
D2_DIRECT_MOD = 8           # dir2: mm % 8 == 0 reduced directly

_CACHE = {}


def _build_nc():
    from contextlib import ExitStack

    import concourse.bacc as bacc
    import concourse.mybir as mybir
    import concourse.tile as tile

    bf16 = mybir.dt.bfloat16
    f32 = mybir.dt.float32
    AX = mybir.AxisListType.X
    MIN = mybir.AluOpType.min

    nc = bacc.Bacc()
    aug = nc.dram_tensor("aug", [P, TOT_COLS], bf16, kind="ExternalInput")
    row_out = nc.dram_tensor("row_out", [P, NS // P], f32, kind="ExternalOutput")
    col_out = nc.dram_tensor("col_out", [P, N // P], f32, kind="ExternalOutput")

    with tile.TileContext(nc) as tc, ExitStack() as ctx:
        sb = ctx.enter_context(tc.tile_pool(name="sb", bufs=1))
        ps = ctx.enter_context(tc.tile_pool(name="ps", bufs=2, space="PSUM"))
        cnvp = ctx.enter_context(tc.tile_pool(name="cnvp", bufs=4))
        runp = ctx.enter_context(tc.tile_pool(name="runp", bufs=3))
        mn = ctx.enter_context(tc.tile_pool(name="mn", bufs=4))
        outp = ctx.enter_context(tc.tile_pool(name="outp", bufs=1))

        aug_sb = sb.tile([P, TOT_COLS], bf16)
        nc.sync.dma_start(out=aug_sb[:, :], in_=aug[:, :])

        row_acc = outp.tile([P, NS // P], f32)
        col_acc = outp.tile([P, N // P], f32)

        def packed_group(pt, w_off, r_off):
            """4 concurrent matmuls (row groups g=0..3) filling pt[128,2048].
            Row group g handles the g-th 512-column sub-slice."""
            for g in range(4):
                bp = KPAD * g
                nc.tensor.matmul(
                    pt[:, g * MM_N:(g + 1) * MM_N],
                    aug_sb[bp:bp + K, w_off:w_off + P],
                    aug_sb[bp:bp + K, r_off + g * MM_N:r_off + (g + 1) * MM_N],
                    start=True,
                    stop=True,
                    tile_position=(bp, 0),
                )

        def fold_to_col(src_bf, dst_col):
            """src_bf [128,2048] bf16 -> min over free axis -> dst_col [128,1]."""
            f1 = runp.tile([P, 1024], bf16, tag="f1")
            nc.vector.tensor_tensor(
                out=f1[:, :], in0=src_bf[:, 0:1024], in1=src_bf[:, 1024:2048], op=MIN
            )
            f2 = runp.tile([P, 512], bf16, tag="f2")
            nc.vector.tensor_tensor(
                out=f2[:, :], in0=f1[:, 0:512], in1=f1[:, 512:1024], op=MIN
            )
            nc.vector.tensor_reduce(dst_col, f2[:, :], axis=AX, op=MIN)

        # ---- direction 1: a-shard rows on partitions, min over all b
        n_chunks = NS // P              # 16
        m_groups = N // GRP             # 8
        for n in range(n_chunks):
            n_direct = len(D1_DIRECT)
            minlets = mn.tile([P, n_direct + 1], f32, tag="minlets")
            run = None
            di = 0
            for mg in range(m_groups):
                pt = ps.tile([P, GRP], f32, tag="pt")
                packed_group(pt, OFF_WA + n * P, OFF_RB + mg * GRP)
                if mg in D1_DIRECT:
                    nc.vector.tensor_reduce(
                        minlets[:, di:di + 1], pt[:, :], axis=AX, op=MIN
                    )
                    di += 1
                else:
                    cnv = cnvp.tile([P, GRP], bf16, tag="cnv")
                    nc.scalar.copy(cnv[:, :], pt[:, :])
                    if run is None:
                        run = cnv
                    else:
                        nrun = runp.tile([P, GRP], bf16, tag="run")
                        nc.vector.tensor_tensor(
                            out=nrun[:, :], in0=run[:, :], in1=cnv[:, :], op=MIN
                        )
                        run = nrun
            fold_to_col(run, minlets[:, n_direct:n_direct + 1])
            nc.vector.tensor_reduce(
                row_acc[:, n:n + 1], minlets[:, :], axis=AX, op=MIN
            )
        nc.sync.dma_start(out=row_out[:, :], in_=row_acc[:, :])

        # ---- direction 2: b rows on partitions, min over this a-shard
        mm_chunks = N // P              # 128
        for mm in range(mm_chunks):
            pt = ps.tile([P, GRP], f32, tag="pt")
            packed_group(pt, OFF_WB + mm * P, OFF_RA)
            if mm % D2_DIRECT_MOD == 0:
                nc.vector.tensor_reduce(
                    col_acc[:, mm:mm + 1], pt[:, :], axis=AX, op=MIN
                )
            else:
                cnv = cnvp.tile([P, GRP], bf16, tag="cnv")
                nc.scalar.copy(cnv[:, :], pt[:, :])
                fold_to_col(cnv, col_acc[:, mm:mm + 1])
        nc.sync.dma_start(out=col_out[:, :], in_=col_acc[:, :])

    nc.compile()
    return nc


def _get_nc():
    if "nc" not in _CACHE:
        _CACHE["nc"] = _build_nc()
    return _CACHE["nc"]


def _install_ntff_hook():
    """The agent image's `antenv` lacks `axon_hooks`; provide it so
    run_bass_kernel_spmd(trace=True) can profile via the axon PJRT .so."""
    import sys

    if "antenv.axon_hooks" in sys.modules:
        return
    try:
        import contextlib
        import ctypes
        import types

        so_path = "/opt/axon/libaxon_pjrt.so"
        lib = ctypes.CDLL(so_path)
        if not hasattr(lib, "axon_start_nrt_profile"):
            return
        lib.axon_start_nrt_profile.argtypes = [
            ctypes.POINTER(ctypes.c_int64),
            ctypes.c_size_t,
        ]
        lib.axon_start_nrt_profile.restype = ctypes.c_int64
        lib.axon_stop_nrt_profile.argtypes = [ctypes.c_char_p]
        lib.axon_stop_nrt_profile.restype = ctypes.c_int64

        @contextlib.contextmanager
        def _hook(output_dir, device_ids):
            import jax

            jax.devices()
            if device_ids:
                ids = (ctypes.c_int64 * len(device_ids))(*device_ids)
                rc = lib.axon_start_nrt_profile(ids, len(device_ids))
            else:
                rc = lib.axon_start_nrt_profile(None, 0)
            if rc != 0:
                raise RuntimeError(f"axon_start_nrt_profile rc={rc}")
            try:
                yield
            finally:
                n = lib.axon_stop_nrt_profile(str(output_dir).encode())
                if n < 0:
                    raise RuntimeError(f"axon_stop_nrt_profile rc={n}")

        mod = types.ModuleType("antenv.axon_hooks")
        mod.get_axon_ntff_profile_hook = lambda: _hook
        mod.set_axon_ntff_profile_hook = lambda h: None
        sys.modules["antenv.axon_hooks"] = mod
    except Exception:
        pass


def _run(in_maps, trace=False):
    from concourse.bass_utils import run_bass_kernel_spmd

    if trace:
        _install_ntff_hook()
    nc = _get_nc()
    res = run_bass_kernel_spmd(
        nc, in_maps, core_ids=list(range(NCORES)), trace=trace
    )
    _CACHE["last_exec_ns"] = res.exec_time_ns
    _CACHE["last_trace"] = res.instructions_and_trace
    return res.results


def _split3(x):
    """fp32 -> three bf16 pieces (returned as fp32 for further math)."""
    import ml_dtypes

    h = x.astype(ml_dtypes.bfloat16).astype(np.float32)
    r = x - h
    m = r.astype(ml_dtypes.bfloat16).astype(np.float32)
    l = (r - m).astype(np.float32)
    return h, m, l


# piece-pair schedule per coordinate: indices into (h, m, l)
_PAIRS = [(0, 0), (0, 1), (1, 0), (0, 2), (2, 0), (1, 1), (1, 2), (2, 1)]


def _build_wr(Pts, Qts, P2, Q2):
    """W from the stationary set (with -2*coords and |P|^2), R from the
    streaming set (coords and |Q|^2), such that W[:, i] . R[:, j] = d2."""
    W = np.zeros((K, Pts.shape[0]), np.float32)
    R = np.zeros((K, Qts.shape[0]), np.float32)
    k = 0
    for d in range(D):
        u = _split3(-2.0 * Pts[:, d])
        v = _split3(Qts[:, d])
        for wp, rp in _PAIRS:
            W[k] = u[wp]
            R[k] = v[rp]
            k += 1
    q2p = _split3(Q2)
    for t in range(3):
        W[k] = 1.0
        R[k] = q2p[t]
        k += 1
    p2p = _split3(P2)
    for t in range(3):
        W[k] = p2p[t]
        R[k] = 1.0
        k += 1
    assert k == K
    return W, R


def kernel(a, b):
    import ml_dtypes
    import os

    a = np.ascontiguousarray(np.asarray(a, dtype=np.float32))
    b = np.ascontiguousarray(np.asarray(b, dtype=np.float32))
    assert a.shape == (N, D) and b.shape == (N, D), (a.shape, b.shape)

    a2 = np.sum(a.astype(np.float64) * a, axis=1).astype(np.float32)
    b2 = np.sum(b.astype(np.float64) * b, axis=1).astype(np.float32)

    Wa, Rb = _build_wr(a, b, a2, b2)   # direction 1: a stationary, b streaming
    Wb, Ra = _build_wr(b, a, b2, a2)   # direction 2: b stationary, a streaming

    trace = bool(int(os.environ.get("CHAMFER_TRACE", "0")))
    in_maps = []
    for r in range(NCORES):
        row = np.zeros((KPAD, TOT_COLS), np.float32)
        row[:K, OFF_WA:OFF_WA + NS] = Wa[:, r * NS:(r + 1) * NS]
        row[:K, OFF_RB:OFF_RB + N] = Rb
        row[:K, OFF_WB:OFF_WB + N] = Wb
        row[:K, OFF_RA:OFF_RA + NS] = Ra[:, r * NS:(r + 1) * NS]
        buf = np.tile(row, (4, 1))          # replicas at partitions 0/32/64/96
        in_maps.append({"aug": buf.astype(ml_dtypes.bfloat16)})
    results = _run(in_maps, trace=trace)

    # row_out[p, n] -> row index i = n*128 + p ; shards in core order
    rows = np.concatenate(
        [results[r]["row_out"].T.reshape(-1) for r in range(NCORES)]
    )
    # col partials: min over cores
    cols = np.min(
        np.stack([results[r]["col_out"].T.reshape(-1) for r in range(NCORES)]),
        axis=0,
    )
    mins_sq = np.concatenate([rows, cols])
    dist = np.sqrt(np.maximum(mins_sq, 0.0))
    return np.asarray(np.mean(dist), dtype=np.float32)
